# revision 21
# baseline (speedup 1.0000x reference)
"""Trainium2 Bass kernel for nn_DiffusionBlock (token + channel distance attention).

Strategy: data-parallel over batch B=8 across the 8 NeuronCores (one batch
element per core, zero collectives). All LN weights, sqrt(tau) and gamma are
folded into the six weight matrices on the host (bf16). Softmax row-constants
cancel, so token logits become  exp(S''/4 - q2''[m]/8)  with a per-partition
bias that feeds the ScalarEngine's fused  func(scale*x + bias).
"""

import math
from contextlib import ExitStack

import ml_dtypes
import numpy as np

import concourse.bass as bass
import concourse.tile as tile
from concourse import bacc
from concourse import mybir
from concourse import bass_utils

F32 = mybir.dt.float32
BF16 = mybir.dt.bfloat16

B = 8
N = 1024
D = 1024
H = 16          # token heads
CH = 8          # channel heads
DH = D // H     # 64
DHC = D // CH   # 128
P = 128
NP = D // P     # 8 chunks
EPS = 1e-5

AF = mybir.ActivationFunctionType
ALU = mybir.AluOpType


def _np_bf16(a):
    return np.ascontiguousarray(a.astype(np.float32)).astype(ml_dtypes.bfloat16)


def build_nc():
    nc = bacc.Bacc("TRN2", target_bir_lowering=False, debug=False)

    # ---- DRAM I/O ----
    x_d = nc.dram_tensor("x", [N, D], F32, kind="ExternalInput").ap()
    w_names = ["wqk_t", "wv_t", "wo_t", "wqk_c", "wv_c", "wo_c"]
    w_d = {nm: nc.dram_tensor(nm, [D, D], BF16, kind="ExternalInput").ap()
           for nm in w_names}
    out_d = nc.dram_tensor("out", [N, D], F32, kind="ExternalOutput").ap()

    # ---- inline constants ----
    eye = np.eye(P, dtype=np.float32)
    id_bf_d = nc.inline_tensor(_np_bf16(eye), name="id_bf").ap()
    id_f32_d = nc.inline_tensor(eye.copy(), name="id_f32").ap()
    # cq2t[p, dc, h] = -1/8 iff head(128*dc + p) == h   (head = d // 64)
    cq2t_np = np.zeros((P, NP, H), dtype=np.float32)
    for dc in range(NP):
        for p in range(P):
            cq2t_np[p, dc, (128 * dc + p) // DH] = -0.125
    cq2t_d = nc.inline_tensor(_np_bf16(cq2t_np), name="cq2t").ap()
    cq2c_d = nc.inline_tensor(
        _np_bf16(np.full((P, 1), -1.0 / 32.0, dtype=np.float32)), name="cq2c").ap()
    ones128_d = nc.inline_tensor(
        _np_bf16(np.ones((P, 1), dtype=np.float32)), name="ones128").ap()

    with tile.TileContext(nc) as tc, ExitStack() as ctx:
        persist = ctx.enter_context(tc.tile_pool(name="persist", bufs=1))
        small = ctx.enter_context(tc.tile_pool(name="small", bufs=1))
        sq_pool = ctx.enter_context(tc.tile_pool(name="sq", bufs=2))
        at_pool = ctx.enter_context(tc.tile_pool(name="at", bufs=3))
        bc_pool = ctx.enter_context(tc.tile_pool(name="bc", bufs=2))
        stat_pool = ctx.enter_context(tc.tile_pool(name="stat", bufs=2))
        ps_mm = ctx.enter_context(tc.tile_pool(name="ps_mm", bufs=2, space="PSUM"))
        ps_acc = ctx.enter_context(tc.tile_pool(name="ps_acc", bufs=2, space="PSUM"))
        dram_pool = ctx.enter_context(tc.tile_pool(name="dscratch", bufs=1, space="DRAM"))

        # ---- persistent SBUF tiles ----
        xres = persist.tile([P, NP, N], F32, tag="xres")          # x, then x+y1
        wt = {nm: persist.tile([P, NP, D], BF16, tag=f"w{i % 3}", name=f"sb_{nm}")
              for i, nm in enumerate(w_names)}
        z = persist.tile([P, NP, D], BF16, tag="z")               # LN output
        zT = persist.tile([P, NP, N], BF16, tag="zT")             # z transposed
        qkT = persist.tile([P, NP, N], BF16, tag="qkT")           # qk'' transposed
        v1 = persist.tile([P, NP, H, DH + 1], BF16, tag="v1")     # v + ones col
        aoT = persist.tile([P, NP, N], BF16, tag="aoT")           # attnout^T

        id_bf = small.tile([P, P], BF16)
        id_f32 = small.tile([P, P], F32)
        cq2t = small.tile([P, NP, H], BF16)
        cq2c = small.tile([P, 1], BF16)
        ones128 = small.tile([P, 1], BF16)
        eps_t = small.tile([P, 1], F32)
        q2s = small.tile([H, N], F32)       # -q2''_h[m]/8  rows
        q2T = small.tile([P, NP, H], F32)   # transposed per-partition bias
        zrows = small.tile([H, N], F32)     # softmax denominators (token)
        rcpzb = small.tile([H, N], BF16)
        q2cs = small.tile([1, D], F32, tag="q2s", padded_shape=[H, N])
        q2cT = small.tile([P, CH], F32)
        zcs = small.tile([CH, DHC], F32)
        rcpzc = small.tile([CH, DHC], F32)
        rcpzcT = small.tile([P, CH], F32)

        nc.vector.memset(eps_t, EPS)
        nc.sync.dma_start(out=id_bf, in_=id_bf_d)
        nc.sync.dma_start(out=id_f32, in_=id_f32_d)
        nc.sync.dma_start(out=cq2t, in_=cq2t_d)
        nc.sync.dma_start(out=cq2c, in_=cq2c_d)
        nc.sync.dma_start(out=ones128, in_=ones128_d)
        for i in range(NP):
            nc.sync.dma_start(out=xres[:, i, :], in_=x_d[i * P:(i + 1) * P, :])
        for nm in w_names:
            for i in range(NP):
                nc.sync.dma_start(out=wt[nm][:, i, :],
                                  in_=w_d[nm][i * P:(i + 1) * P, :])

        # ---------------- helpers ----------------
        def layer_norm(src, dst):
            """dst[:, i, :] (bf16) = (src - mean)/sqrt(var+eps), per row."""
            mv = stat_pool.tile([P, NP, 2], F32, tag="mv", bufs=1)
            for i in range(NP):
                stats = stat_pool.tile([P, 2, 6], F32, tag="stats")
                nc.vector.bn_stats(out=stats[:, 0, :], in_=src[:, i, 0:512])
                nc.vector.bn_stats(out=stats[:, 1, :], in_=src[:, i, 512:1024])
                nc.vector.bn_aggr(out=mv[:, i, :], in_=stats)
            std = stat_pool.tile([P, NP], F32, tag="std", bufs=1)
            rsig = stat_pool.tile([P, NP], F32, tag="rsig", bufs=1)
            nc.scalar.activation(out=std, in_=mv[:, :, 1], func=AF.Sqrt,
                                 bias=eps_t, scale=1.0)
            nc.vector.reciprocal(out=rsig, in_=std)
            for i in range(NP):
                nc.vector.tensor_scalar(
                    out=dst[:, i, :], in0=src[:, i, :],
                    scalar1=mv[:, i, 0:1], scalar2=rsig[:, i:i + 1],
                    op0=ALU.subtract, op1=ALU.mult)

        def transpose_full(src, dst):
            """dst = src^T for a [1024, 1024] bf16 tensor held as chunk tiles."""
            for i in range(NP):
                pt = ps_mm.tile([P, N], BF16, tag="m", padded_shape=[P, N])
                for j in range(NP):
                    nc.tensor.transpose(pt[:, j * P:(j + 1) * P],
                                        src[:, j, i * P:(i + 1) * P], id_bf)
                nc.vector.tensor_copy(out=dst[:, i, :], in_=pt)

        def proj_T(w, dst):
            """dst = (z @ W)^T: dst[:, dc, n] over output chunks dc."""
            for dc in range(NP):
                pm = ps_acc.tile([P, N], F32, name="pm_projT", tag="a", padded_shape=[P, N])
                for dk in range(NP):
                    lhsT = w[:, dk, dc * P:(dc + 1) * P]
                    for hf in range(2):
                        nc.tensor.matmul(pm[:, hf * 512:(hf + 1) * 512], lhsT,
                                         zT[:, dk, hf * 512:(hf + 1) * 512],
                                         start=(dk == 0), stop=(dk == NP - 1))
                nc.vector.tensor_copy(out=dst[:, dc, :], in_=pm)

        def proj_N(w, consume):
            """natural-layout projection: psum[n-chunk, dout] -> consume(ni, pm)"""
            for ni in range(NP):
                pm = ps_acc.tile([P, D], F32, name="pm_projN", tag="a", padded_shape=[P, N])
                for dk in range(NP):
                    lhsT = zT[:, dk, ni * P:(ni + 1) * P]
                    for hf in range(2):
                        nc.tensor.matmul(pm[:, hf * 512:(hf + 1) * 512], lhsT,
                                         w[:, dk, hf * 512:(hf + 1) * 512],
                                         start=(dk == 0), stop=(dk == NP - 1))
                consume(ni, pm)

        # ================= block 1: token diffusion =================
        layer_norm(xres, z)
        transpose_full(z, zT)
        proj_T(wt["wqk_t"], qkT)

        # q2 rows: q2s[h, m] = -(1/8) * sum_{d in head h} qk''[m, d]^2
        q2ps = ps_acc.tile([H, N], F32, name="q2ps", tag="a", padded_shape=[P, N])
        for dc in range(NP):
            sq = sq_pool.tile([P, N], BF16, name="sq_t", tag="sq")
            nc.vector.tensor_mul(out=sq, in0=qkT[:, dc, :], in1=qkT[:, dc, :])
            for hf in range(2):
                nc.tensor.matmul(q2ps[:, hf * 512:(hf + 1) * 512], cq2t[:, dc, :],
                                 sq[:, hf * 512:(hf + 1) * 512],
                                 start=(dc == 0), stop=(dc == NP - 1))
        nc.scalar.copy(out=q2s, in_=q2ps)
        for k in range(NP):
            pt = ps_mm.tile([P, H], F32, name="q2t_ps", tag="m", padded_shape=[P, N])
            nc.tensor.transpose(pt, q2s[:, k * P:(k + 1) * P], id_f32[0:H, 0:H])
            nc.vector.tensor_copy(out=q2T[:, k, :], in_=pt)

        def consume_v(ni, pm):
            nc.vector.tensor_copy(
                out=v1[:, ni, :, 0:DH],
                in_=pm.rearrange("p (h c) -> p h c", h=H))
            nc.vector.memset(v1[:, ni, :, DH:DH + 1], 1.0)
        proj_N(wt["wv_t"], consume_v)

        # per-pair token attention (heads 2*pr, 2*pr+1 live in qkT chunk pr)
        for pr in range(NP):
            av = [ps_acc.tile([P, N], F32, name=f"av{s}", tag="a", padded_shape=[P, N]) for s in range(2)]
            for mc in range(NP):
                for s in range(2):
                    h = 2 * pr + s
                    st = ps_mm.tile([P, N], F32, name="st", tag="m", padded_shape=[P, N])
                    lhsT = qkT[64 * s:64 * (s + 1), pr, mc * P:(mc + 1) * P]
                    for hf in range(2):
                        nc.tensor.matmul(
                            st[:, hf * 512:(hf + 1) * 512], lhsT,
                            qkT[64 * s:64 * (s + 1), pr, hf * 512:(hf + 1) * 512],
                            start=True, stop=True, tile_position=(64 * s, 0))
                    aT = at_pool.tile([P, N], BF16, name="aT")
                    nc.scalar.activation(out=aT, in_=st, func=AF.Exp,
                                         bias=q2T[:, mc, h:h + 1], scale=0.25)
                    for hf in range(2):
                        nc.tensor.matmul(
                            av[s][0:DH + 1, hf * 512:(hf + 1) * 512],
                            v1[:, mc, h, 0:DH + 1],
                            aT[:, hf * 512:(hf + 1) * 512],
                            start=(mc == 0), stop=(mc == NP - 1))
            for s in range(2):
                h = 2 * pr + s
                nc.vector.tensor_copy(out=aoT[64 * s:64 * (s + 1), pr, :],
                                      in_=av[s][0:DH, :])
                zr_tmp = bc_pool.tile([1, N], F32, tag="zr_tmp", name="zr_tmp")
                nc.scalar.copy(out=zr_tmp, in_=av[s][DH:DH + 1, :])
                nc.sync.dma_start(out=zrows[h:h + 1, :], in_=zr_tmp)

        # normalize: aoT[d, n] *= 1/Z[head(d), n]
        nc.vector.reciprocal(out=zrows, in_=zrows)
        nc.vector.tensor_copy(out=rcpzb, in_=zrows)
        rz_d = dram_pool.tile([H, N], BF16, name="rz_d")
        nc.sync.dma_start(out=rz_d, in_=rcpzb)
        for pr in range(NP):
            bcst = bc_pool.tile([P, N], BF16, name="bcst")
            for s in range(2):
                h = 2 * pr + s
                nc.sync.dma_start(out=bcst[64 * s:64 * (s + 1), :],
                                  in_=rz_d[h:h + 1, :].to_broadcast((DH, N)))
            nc.vector.tensor_mul(out=aoT[:, pr, :], in0=aoT[:, pr, :], in1=bcst)

        # y1 = attnout @ Wo_t' ; x <- x + y1  (in place)
        for ni in range(NP):
            pm = ps_acc.tile([P, D], F32, name="pm_y1", tag="a", padded_shape=[P, N])
            for dk in range(NP):
                lhsT = aoT[:, dk, ni * P:(ni + 1) * P]
                for hf in range(2):
                    nc.tensor.matmul(pm[:, hf * 512:(hf + 1) * 512], lhsT,
                                     wt["wo_t"][:, dk, hf * 512:(hf + 1) * 512],
                                     start=(dk == 0), stop=(dk == NP - 1))
            nc.vector.tensor_add(out=xres[:, ni, :], in0=xres[:, ni, :], in1=pm)

        # ================= block 2: channel diffusion =================
        z2 = persist.tile([P, NP, D], BF16, tag="z")
        z2T = persist.tile([P, NP, N], BF16, tag="zT")
        qkc = persist.tile([P, NP, D], BF16, tag="qkT")    # natural layout
        vcT = persist.tile([P, NP, N], BF16, tag="v1")     # transposed layout
        aocT = persist.tile([P, CH, N], BF16, tag="aoT")

        layer_norm(xres, z2)
        transpose_full(z2, z2T)

        def consume_qkc(ni, pm):
            nc.vector.tensor_copy(out=qkc[:, ni, :], in_=pm)
        # NB: proj_N/proj_T read zT which now aliases z2T via tag sharing
        def proj_N2(w, consume):
            for ni in range(NP):
                pm = ps_acc.tile([P, D], F32, name="pm_projN2", tag="a", padded_shape=[P, N])
                for dk in range(NP):
                    lhsT = z2T[:, dk, ni * P:(ni + 1) * P]
                    for hf in range(2):
                        nc.tensor.matmul(pm[:, hf * 512:(hf + 1) * 512], lhsT,
                                         w[:, dk, hf * 512:(hf + 1) * 512],
                                         start=(dk == 0), stop=(dk == NP - 1))
                consume(ni, pm)
        proj_N2(wt["wqk_c"], consume_qkc)

        # q2c row: q2cs[0, j] = -(1/32) sum_n qkc[n, j]^2
        q2cps = ps_acc.tile([1, D], F32, name="q2cps", tag="a", padded_shape=[P, N])
        for ni in range(NP):
            sqc = sq_pool.tile([P, D], BF16, name="sq_c", tag="sq")
            nc.vector.tensor_mul(out=sqc, in0=qkc[:, ni, :], in1=qkc[:, ni, :])
            for hf in range(2):
                nc.tensor.matmul(q2cps[:, hf * 512:(hf + 1) * 512], cq2c,
                                 sqc[:, hf * 512:(hf + 1) * 512],
                                 start=(ni == 0), stop=(ni == NP - 1))
        nc.scalar.copy(out=q2cs, in_=q2cps)
        for k in range(NP):
            pt = ps_mm.tile([P, 1], F32, name="q2ct_ps", tag="m", padded_shape=[P, N])
            nc.tensor.transpose(pt, q2cs[:, k * P:(k + 1) * P], id_f32[0:1, 0:1])
            nc.vector.tensor_copy(out=q2cT[:, k:k + 1], in_=pt)

        for dc in range(NP):
            pm = ps_acc.tile([P, N], F32, name="pm_vcT", tag="a", padded_shape=[P, N])
            for dk in range(NP):
                lhsT = wt["wv_c"][:, dk, dc * P:(dc + 1) * P]
                for hf in range(2):
                    nc.tensor.matmul(pm[:, hf * 512:(hf + 1) * 512], lhsT,
                                     z2T[:, dk, hf * 512:(hf + 1) * 512],
                                     start=(dk == 0), stop=(dk == NP - 1))
            nc.vector.tensor_copy(out=vcT[:, dc, :], in_=pm)

        # channel attention (head h == chunk h since dh_c = 128)
        for h in range(CH):
            sc = ps_acc.tile([P, DHC], F32, name="sc", tag="a", padded_shape=[P, N])
            for k in range(NP):
                nc.tensor.matmul(sc, qkc[:, k, h * P:(h + 1) * P],
                                 qkc[:, k, h * P:(h + 1) * P],
                                 start=(k == 0), stop=(k == NP - 1))
            acT = at_pool.tile([P, DHC], BF16, tag="acT", name="acT")
            nc.scalar.activation(out=acT, in_=sc, func=AF.Exp,
                                 bias=q2cT[:, h:h + 1], scale=1.0 / 16.0)
            oc = ps_acc.tile([P, N], F32, name="oc", tag="a", padded_shape=[P, N])
            for hf in range(2):
                nc.tensor.matmul(oc[:, hf * 512:(hf + 1) * 512], acT,
                                 vcT[:, h, hf * 512:(hf + 1) * 512],
                                 start=True, stop=True)
            zc = ps_mm.tile([1, DHC], F32, name="zc", tag="m", padded_shape=[P, N])
            nc.tensor.matmul(zc, ones128, acT, start=True, stop=True)
            nc.scalar.copy(out=aocT[:, h, :], in_=oc)
            zc_tmp = bc_pool.tile([1, DHC], F32, tag="zc_tmp", name="zc_tmp")
            nc.scalar.copy(out=zc_tmp, in_=zc)
            nc.sync.dma_start(out=zcs[h:h + 1, :], in_=zc_tmp)

        nc.vector.reciprocal(out=rcpzc, in_=zcs)
        ptc = ps_mm.tile([P, CH], F32, name="ptc", tag="m", padded_shape=[P, N])
        nc.tensor.transpose(ptc, rcpzc, id_f32[0:CH, 0:CH])
        nc.vector.tensor_copy(out=rcpzcT, in_=ptc)
        for h in range(CH):
            nc.vector.tensor_scalar_mul(out=aocT[:, h, :], in0=aocT[:, h, :],
                                        scalar1=rcpzcT[:, h:h + 1])

        # y2 = attnout_c @ Wo_c' ; out = x' + y2
        for ni in range(NP):
            pm = ps_acc.tile([P, D], F32, name="pm_y2", tag="a", padded_shape=[P, N])
            for dk in range(NP):
                lhsT = aocT[:, dk, ni * P:(ni + 1) * P]
                for hf in range(2):
                    nc.tensor.matmul(pm[:, hf * 512:(hf + 1) * 512], lhsT,
                                     wt["wo_c"][:, dk, hf * 512:(hf + 1) * 512],
                                     start=(dk == 0), stop=(dk == NP - 1))
            nc.vector.tensor_add(out=xres[:, ni, :], in0=xres[:, ni, :], in1=pm)
            nc.sync.dma_start(out=out_d[ni * P:(ni + 1) * P, :], in_=xres[:, ni, :])

    nc.compile()
    return nc


def fold_weights(Wqk_t, Wv_t, Wo_t, tau_t, Wqk_c, Wv_c, Wo_c, tau_c,
                 ln1_w, ln1_b, ln2_w, ln2_b, gamma1, gamma2):
    tau_t = np.asarray(tau_t, dtype=np.float64).reshape(H)
    tau_c = np.asarray(tau_c, dtype=np.float64).reshape(CH)
    assert (tau_t >= 0).all() and (tau_c >= 0).all(), "negative tau unsupported"
    assert not np.any(np.asarray(ln1_b)) and not np.any(np.asarray(ln2_b)), \
        "nonzero LN bias unsupported"
    st = np.repeat(np.sqrt(tau_t), DH)           # per-column scale (token)
    sc = np.repeat(np.sqrt(tau_c), DHC)          # per-column scale (channel)
    f64 = lambda a: np.asarray(a, dtype=np.float64)
    w = {
        "wqk_t": f64(ln1_w)[:, None] * f64(Wqk_t) * st[None, :],
        "wv_t": f64(ln1_w)[:, None] * f64(Wv_t),
        "wo_t": f64(Wo_t) * f64(gamma1)[None, :],
        "wqk_c": f64(ln2_w)[:, None] * f64(Wqk_c) * sc[None, :],
        "wv_c": f64(ln2_w)[:, None] * f64(Wv_c),
        "wo_c": f64(Wo_c) * f64(gamma2)[None, :],
    }
    return {k: _np_bf16(v) for k, v in w.items()}


def kernel(x, Wqk_t, Wv_t, Wo_t, tau_t, Wqk_c, Wv_c, Wo_c, tau_c,
           ln1_w, ln1_b, ln2_w, ln2_b, gamma1, gamma2):
    x = np.asarray(x, dtype=np.float32)
    wf = fold_weights(Wqk_t, Wv_t, Wo_t, tau_t, Wqk_c, Wv_c, Wo_c, tau_c,
                      ln1_w, ln1_b, ln2_w, ln2_b, gamma1, gamma2)
    nc = build_nc()
    in_maps = [{"x": np.ascontiguousarray(x[b]), **wf} for b in range(B)]
    res = bass_utils.run_bass_kernel_spmd(nc, in_maps, core_ids=list(range(B)))
    return np.stack([res.results[b]["out"] for b in range(B)]).astype(np.float32)
